# revision 2
# baseline (speedup 1.0000x reference)
"""KAN EncoderNetwork kernel for 8 Trainium2 NeuronCores.

Strategy (data-parallel, batch sharded 8 ways, weights replicated):

Each KAN layer  out = silu(x) @ sb + einsum('big,iog->bo', B(x), coef*ss)
is reformulated as ONE matmul per layer over an expanded feature matrix:

  out^T[o,b] = sum_K W'[K,o] * F[K,b]

where for every 128-wide input chunk the feature rows are 8 spline basis
blocks + 1 silu block (9*din rows total).  The uniform-grid cubic
B-spline basis has the closed form (cardinal spline, t = 2.5x + 5.5):

  6*B_g(x) = relu(2-w)^3 - 4*relu(1-w)^3,   w = |2.5x + 3.5 - g|

computed with 1 ScalarE Abs + 2 custom VectorE ops per basis function.
Everything stays feature-major ([feat, batch]) so layer outputs in PSUM
feed the next layer's basis computation directly; only the final layer
is transposed back (TensorE transposes) for the [batch, out] output.

Weights are pre-assembled host-side into bf16 W' matrices with rows
ordered (in_chunk, block g in 0..8, lane) matching the feature layout.
"""

import sys

sys.path.insert(0, "/opt/trn_rl_repo")

import numpy as np
import ml_dtypes

import concourse.bacc as bacc
import concourse.mybir as mybir
import concourse.tile as tile
from concourse.bass_utils import run_bass_kernel_spmd
from concourse.masks import make_identity
from concourse.dve_spec import Spec, Src0, Src1, C0, C1, relu, sq, lower, _has_src1
from concourse.dve_uop import DveOpSpec
from concourse.dve_ops import (
    DveOp,
    OPS,
    _SUB_OPCODE_FOR_NAME,
    CUSTOM_DVE_SPECS,
    _CUSTOM_DVE_ROW_BASE,
)

F32 = mybir.dt.float32
BF16 = mybir.dt.bfloat16
AF = mybir.ActivationFunctionType

WIDTH = [512, 1024, 1024, 1024, 256]
NCORES = 8
BATCH = 4096
BPC = BATCH // NCORES  # 512 batch rows per core
NG = 8  # spline basis functions per input dim
NB = NG + 1  # feature blocks per 128-chunk (8 basis + 1 silu)


def _register_op(name, spec):
    if name in _SUB_OPCODE_FOR_NAME:
        for op in OPS:
            if op.name == name:
                return op
        raise RuntimeError(f"opcode row taken but op {name} missing")
    row = _CUSTOM_DVE_ROW_BASE + len(OPS)
    _SUB_OPCODE_FOR_NAME[name] = row
    shas = {}
    for ver in ("v3", "v4"):
        uops = lower(spec, ver=ver)
        shas[ver] = DveOpSpec(
            name=name, opcode=row, uops=uops, rd1_en=_has_src1(spec)
        ).sha(ver)
    op = DveOp(name, spec, subdim=False, uops_sha=shas)
    OPS.append(op)
    CUSTOM_DVE_SPECS[name] = spec
    return op


# q = relu(s0 - w)^3
_a = relu(C0 - Src0)
KAN_CUBE_TENT = _register_op(
    "KAN_CUBE_TENT",
    Spec(
        body=sq(_a) * _a,
        reference=lambda in0, in1, s0, s1, imm2: np.maximum(s0 - in0, 0.0) ** 3,
    ),
)

# out = q + s1 * relu(s0 - w)^3   (in0=q, in1=w)
_r = relu(C0 - Src1)
KAN_SPLINE_COMBINE = _register_op(
    "KAN_SPLINE_COMBINE",
    Spec(
        body=Src0 + sq(_r) * _r * C1,
        reference=lambda in0, in1, s0, s1, imm2: in0
        + s1 * np.maximum(s0 - in1, 0.0) ** 3,
    ),
)


def _build_nc():
    nc = bacc.Bacc(trn_type="TRN2")
    xT_dr = nc.dram_tensor("xT", [WIDTH[0], BPC], F32, kind="ExternalInput")
    w_dr = [
        nc.dram_tensor(f"w{l}", [NB * WIDTH[l], WIDTH[l + 1]], BF16,
                       kind="ExternalInput")
        for l in range(4)
    ]
    out_dr = nc.dram_tensor("out", [BPC, WIDTH[4]], F32, kind="ExternalOutput")

    with tile.TileContext(nc) as tc:
        with (
            tc.tile_pool(name="const", bufs=1) as const_pool,
            tc.tile_pool(name="xt", bufs=2) as xt_pool,
            tc.tile_pool(name="ft", bufs=3) as ft_pool,
            tc.tile_pool(name="wt", bufs=8) as wt_pool,
            tc.tile_pool(name="tmp", bufs=4) as tmp_pool,
            tc.tile_pool(name="outp", bufs=1) as out_pool,
            tc.tile_pool(name="psum", bufs=8, space="PSUM") as psum_pool,
        ):
            # per-basis Abs bias constants: col g = 3.5 - g
            bias = const_pool.tile([128, NG], F32, tag="bias")
            for g in range(NG):
                nc.gpsimd.memset(bias[:, g : g + 1], 3.5 - g)
            ident = const_pool.tile([128, 128], F32, tag="ident")
            make_identity(nc, ident)

            nic0 = WIDTH[0] // 128
            xt = xt_pool.tile([128, nic0, BPC], F32, tag="xt")
            nc.sync.dma_start(xt, xT_dr.rearrange("(c p) b -> p c b", p=128))

            for l in range(4):
                din, dout = WIDTH[l], WIDTH[l + 1]
                nic, noc = din // 128, dout // 128
                KB = NB * nic
                psums = [
                    psum_pool.tile([128, BPC], F32, tag="psum", name=f"ps_{l}_{i}")
                    for i in range(noc)
                ]
                kb = 0
                for ic in range(nic):
                    xa = xt[:, ic, :]
                    ft = ft_pool.tile([128, NB, BPC], BF16, tag="ft")
                    for g in range(NG):
                        wv = tmp_pool.tile([128, BPC], F32, tag="wv")
                        nc.scalar.activation(wv, xa, AF.Abs,
                                             bias=bias[:, g : g + 1], scale=2.5)
                        qv = tmp_pool.tile([128, BPC], F32, tag="qv")
                        nc.vector._custom_dve(KAN_CUBE_TENT, out=qv, in0=wv,
                                              s0=2.0)
                        nc.vector._custom_dve(KAN_SPLINE_COMBINE,
                                              out=ft[:, g, :], in0=qv, in1=wv,
                                              s0=1.0, s1=-4.0)
                    nc.scalar.activation(ft[:, NG, :], xa, AF.Silu)
                    for g in range(NB):
                        wt = wt_pool.tile([128, dout], BF16, tag="wt")
                        nc.sync.dma_start(
                            wt, w_dr[l][kb * 128 : (kb + 1) * 128, :]
                        )
                        for oc in range(noc):
                            nc.tensor.matmul(
                                psums[oc],
                                wt[:, oc * 128 : (oc + 1) * 128],
                                ft[:, g, :],
                                start=(kb == 0),
                                stop=(kb == KB - 1),
                            )
                        kb += 1
                if l < 3:
                    xt = xt_pool.tile([128, noc, BPC], F32, tag="xt")
                    for oc in range(noc):
                        nc.scalar.copy(xt[:, oc, :], psums[oc])
                else:
                    s3 = out_pool.tile([128, noc, BPC], F32, tag="s3")
                    for oc in range(noc):
                        nc.scalar.copy(s3[:, oc, :], psums[oc])
                    outT = out_pool.tile([128, BPC // 128, WIDTH[4]], F32,
                                         tag="outT")
                    for j in range(BPC // 128):
                        for oc in range(noc):
                            pst = psum_pool.tile([128, 128], F32, tag="psum", name=f"pst_{j}_{oc}")
                            nc.tensor.transpose(
                                pst, s3[:, oc, j * 128 : (j + 1) * 128], ident
                            )
                            nc.vector.tensor_copy(
                                outT[:, j, oc * 128 : (oc + 1) * 128], pst
                            )
                    nc.sync.dma_start(
                        out_dr.rearrange("(j p) o -> p j o", p=128), outT
                    )
    nc.finalize()
    return nc


_NC_CACHE = []


def _get_nc():
    if not _NC_CACHE:
        _NC_CACHE.append(_build_nc())
    return _NC_CACHE[0]


def _build_weights(inp):
    ws = {}
    for l in range(4):
        din, dout = WIDTH[l], WIDTH[l + 1]
        coef = np.asarray(inp[f"coef{l}"], dtype=np.float32)
        sb = np.asarray(inp[f"sb{l}"], dtype=np.float32)
        ss = np.asarray(inp[f"ss{l}"], dtype=np.float32)
        spline_w = coef * ss[:, :, None] * (1.0 / 6.0)  # [din, dout, 8]
        nic = din // 128
        sp = spline_w.reshape(nic, 128, dout, NG).transpose(0, 3, 1, 2)
        base = sb.reshape(nic, 128, dout)[:, None]
        W = np.concatenate([sp, base], axis=1).reshape(nic * NB * 128, dout)
        ws[f"w{l}"] = np.ascontiguousarray(W).astype(ml_dtypes.bfloat16)
    return ws


def _run(inputs, trace=False, **kwargs):
    inp = {k: np.asarray(v) for k, v in inputs.items()}
    ws = _build_weights(inp)
    x = np.concatenate(
        [inp["inputs_y"].astype(np.float32), inp["inputs_u"].astype(np.float32)],
        axis=1,
    )
    xT = np.ascontiguousarray(x.T)  # [512 feat, 4096 batch]
    nc = _get_nc()
    in_maps = []
    for c in range(NCORES):
        m = {"xT": np.ascontiguousarray(xT[:, c * BPC : (c + 1) * BPC])}
        m.update(ws)
        in_maps.append(m)
    res = run_bass_kernel_spmd(
        nc, in_maps, core_ids=list(range(NCORES)), trace=trace, **kwargs
    )
    out = np.concatenate([r["out"] for r in res.results], axis=0)
    return out.astype(np.float32), res


def kernel(**inputs) -> np.ndarray:
    out, _ = _run(inputs)
    return out


# revision 3
# speedup vs baseline: 1.0332x; 1.0332x over previous
"""KAN EncoderNetwork kernel for 8 Trainium2 NeuronCores.

Strategy (data-parallel, batch sharded 8 ways, weights replicated):

Each KAN layer  out = silu(x) @ sb + einsum('big,iog->bo', B(x), coef*ss)
is reformulated as ONE matmul per layer over an expanded feature matrix:

  out^T[o,b] = sum_K W'[K,o] * F[K,b]

where for every 128-wide input chunk the feature rows are 8 spline basis
blocks + 1 silu block (9*din rows total).  The uniform-grid cubic
B-spline basis has the closed form (cardinal spline, t = 2.5x + 5.5):

  6*B_g(x) = relu(2-w)^3 - 4*relu(1-w)^3,   w = |2.5x + 3.5 - g|

computed on ScalarE (Abs/Relu) + custom VectorE ops, balanced across the
two engines.  Everything stays feature-major ([feat, batch]) so layer
outputs in PSUM feed the next layer's basis computation directly; only
the final layer is transposed back (TensorE) for the [batch, out] output.

Weights are pre-assembled host-side into bf16 W' matrices with rows
ordered (in_chunk, block g in 0..8, lane) matching the feature layout.
"""

import sys

sys.path.insert(0, "/opt/trn_rl_repo")

import numpy as np
import ml_dtypes

import concourse.bacc as bacc
import concourse.mybir as mybir
import concourse.tile as tile
from concourse.bass_utils import run_bass_kernel_spmd
from concourse.masks import make_identity
from concourse.dve_spec import Spec, Src0, Src1, C0, C1, relu, sq, lower, _has_src1
from concourse.dve_uop import DveOpSpec
from concourse.dve_ops import (
    DveOp,
    OPS,
    _SUB_OPCODE_FOR_NAME,
    CUSTOM_DVE_SPECS,
    _CUSTOM_DVE_ROW_BASE,
)

F32 = mybir.dt.float32
BF16 = mybir.dt.bfloat16
AF = mybir.ActivationFunctionType

WIDTH = [512, 1024, 1024, 1024, 256]
NCORES = 8
BATCH = 4096
BPC = BATCH // NCORES  # 512 batch rows per core
NG = 8  # spline basis functions per input dim
NB = NG + 1  # feature blocks per 128-chunk (8 basis + 1 silu)

# which basis functions use the ACT-heavy pipeline (B) vs DVE-heavy (A)
VARIANT_B = {5, 6, 7}


def _register_op(name, spec):
    if name in _SUB_OPCODE_FOR_NAME:
        for op in OPS:
            if op.name == name:
                return op
        raise RuntimeError(f"opcode row taken but op {name} missing")
    row = _CUSTOM_DVE_ROW_BASE + len(OPS)
    _SUB_OPCODE_FOR_NAME[name] = row
    shas = {}
    for ver in ("v3", "v4"):
        uops = lower(spec, ver=ver)
        shas[ver] = DveOpSpec(
            name=name, opcode=row, uops=uops, rd1_en=_has_src1(spec)
        ).sha(ver)
    op = DveOp(name, spec, subdim=False, uops_sha=shas)
    OPS.append(op)
    CUSTOM_DVE_SPECS[name] = spec
    return op


# q = relu(s0 - w)^3        (variant A, pass 1; 1 stream)
_a = relu(C0 - Src0)
KAN_CUBE_TENT = _register_op(
    "KAN_CUBE_TENT",
    Spec(
        body=sq(_a) * _a,
        reference=lambda in0, in1, s0, s1, imm2: np.maximum(s0 - in0, 0.0) ** 3,
    ),
)

# out = q + s1 * relu(s0 - w)^3   (variant A, pass 2; in0=q, in1=w; 2 streams)
_r = relu(C0 - Src1)
KAN_SPLINE_COMBINE = _register_op(
    "KAN_SPLINE_COMBINE",
    Spec(
        body=Src0 + sq(_r) * _r * C1,
        reference=lambda in0, in1, s0, s1, imm2: in0
        + s1 * np.maximum(s0 - in1, 0.0) ** 3,
    ),
)

# out = a^3 + s1 * relu(a - s0)^3   (variant B; in0 = a2 = relu(2-w); 1 stream)
_rb = relu(Src0 - C0)
KAN_TENT_POLY = _register_op(
    "KAN_TENT_POLY",
    Spec(
        body=sq(Src0) * Src0 + sq(_rb) * _rb * C1,
        reference=lambda in0, in1, s0, s1, imm2: in0**3
        + s1 * np.maximum(in0 - s0, 0.0) ** 3,
    ),
)


def _chunk_groups(nic):
    """Basis-op batching: keep the first two chunks solo (short critical
    chain at layer boundaries), pair the rest."""
    groups = [[0]]
    if nic >= 2:
        groups.append([1])
    c = 2
    while c < nic:
        groups.append(list(range(c, min(c + 2, nic))))
        c += 2
    return groups


def _build_nc():
    nc = bacc.Bacc(trn_type="TRN2")
    xT_dr = nc.dram_tensor("xT", [WIDTH[0], BPC], F32, kind="ExternalInput")
    w_dr = [
        nc.dram_tensor(f"w{l}", [NB * WIDTH[l], WIDTH[l + 1]], BF16,
                       kind="ExternalInput")
        for l in range(4)
    ]
    out_dr = nc.dram_tensor("out", [BPC, WIDTH[4]], F32, kind="ExternalOutput")

    with tile.TileContext(nc) as tc:
        with (
            tc.tile_pool(name="const", bufs=1) as const_pool,
            tc.tile_pool(name="xt", bufs=2) as xt_pool,
            tc.tile_pool(name="ft", bufs=4) as ft_pool,
            tc.tile_pool(name="wt", bufs=8) as wt_pool,
            tc.tile_pool(name="tmp", bufs=4) as tmp_pool,
            tc.tile_pool(name="outp", bufs=1) as out_pool,
            tc.tile_pool(name="psum", bufs=8, space="PSUM") as psum_pool,
        ):
            # col g in 0..7: Abs bias 3.5-g ; col 8: +2.0 (variant-B Relu bias)
            bias = const_pool.tile([128, NB], F32, tag="bias")
            for g in range(NG):
                nc.gpsimd.memset(bias[:, g : g + 1], 3.5 - g)
            nc.gpsimd.memset(bias[:, NG : NG + 1], 2.0)
            ident = const_pool.tile([128, 128], F32, tag="ident")
            make_identity(nc, ident)

            nic0 = WIDTH[0] // 128
            xt = xt_pool.tile([128, nic0, BPC], F32, tag="xt")
            xT_r = xT_dr.rearrange("(c p) b -> p c b", p=128)
            for c in range(nic0):
                nc.sync.dma_start(xt[:, c : c + 1, :], xT_r[:, c : c + 1, :])

            for l in range(4):
                din, dout = WIDTH[l], WIDTH[l + 1]
                nic, noc = din // 128, dout // 128
                KB = NB * nic
                psums = [
                    psum_pool.tile([128, BPC], F32, tag="psum", name=f"ps_{l}_{i}")
                    for i in range(noc)
                ]
                for group in _chunk_groups(nic):
                    i0, s = group[0], len(group)
                    W = s * BPC
                    xa = xt[:, i0 : i0 + s, :].rearrange("p c b -> p (c b)")
                    ft = ft_pool.tile([128, NB, W], BF16, tag="ft",
                                      name=f"ft_{l}_{i0}")
                    for g in range(NG):
                        wv = tmp_pool.tile([128, W], F32, tag="wv",
                                           name=f"wv_{l}_{i0}_{g}")
                        nc.scalar.activation(wv, xa, AF.Abs,
                                             bias=bias[:, g : g + 1], scale=2.5)
                        if g in VARIANT_B:
                            a2 = tmp_pool.tile([128, W], F32, tag="qv",
                                               name=f"a2_{l}_{i0}_{g}")
                            nc.scalar.activation(a2, wv, AF.Relu,
                                                 bias=bias[:, NG : NG + 1],
                                                 scale=-1.0)
                            nc.vector._custom_dve(KAN_TENT_POLY,
                                                  out=ft[:, g, :], in0=a2,
                                                  s0=1.0, s1=-4.0)
                        else:
                            qv = tmp_pool.tile([128, W], F32, tag="qv",
                                               name=f"qv_{l}_{i0}_{g}")
                            nc.vector._custom_dve(KAN_CUBE_TENT, out=qv,
                                                  in0=wv, s0=2.0)
                            nc.vector._custom_dve(KAN_SPLINE_COMBINE,
                                                  out=ft[:, g, :], in0=qv,
                                                  in1=wv, s0=1.0, s1=-4.0)
                    nc.scalar.activation(ft[:, NG, :], xa, AF.Silu)
                    # matmuls for this group's K-blocks
                    for ci, c in enumerate(group):
                        for g in range(NB):
                            kb = (i0 + ci) * NB + g
                            wt = wt_pool.tile([128, dout], BF16, tag="wt",
                                              name=f"wt_{l}_{kb}")
                            nc.sync.dma_start(
                                wt, w_dr[l][kb * 128 : (kb + 1) * 128, :]
                            )
                            rhs = ft[:, g, ci * BPC : (ci + 1) * BPC]
                            for oc in range(noc):
                                nc.tensor.matmul(
                                    psums[oc],
                                    wt[:, oc * 128 : (oc + 1) * 128],
                                    rhs,
                                    start=(kb == 0),
                                    stop=(kb == KB - 1),
                                )
                if l < 3:
                    xt = xt_pool.tile([128, noc, BPC], F32, tag="xt",
                                      name=f"xt_{l}")
                    for oc in range(noc):
                        nc.scalar.copy(xt[:, oc, :], psums[oc])
                else:
                    s3 = out_pool.tile([128, noc, BPC], F32, tag="s3")
                    for oc in range(noc):
                        nc.scalar.copy(s3[:, oc, :], psums[oc])
                    outT = out_pool.tile([128, BPC // 128, WIDTH[4]], F32,
                                         tag="outT")
                    for j in range(BPC // 128):
                        for oc in range(noc):
                            pst = psum_pool.tile([128, 128], F32, tag="psum",
                                                 name=f"pst_{j}_{oc}")
                            nc.tensor.transpose(
                                pst, s3[:, oc, j * 128 : (j + 1) * 128], ident
                            )
                            nc.vector.tensor_copy(
                                outT[:, j, oc * 128 : (oc + 1) * 128], pst
                            )
                    nc.sync.dma_start(
                        out_dr.rearrange("(j p) o -> p j o", p=128), outT
                    )
    nc.finalize()
    return nc


_NC_CACHE = []


def _get_nc():
    if not _NC_CACHE:
        _NC_CACHE.append(_build_nc())
    return _NC_CACHE[0]


def _build_weights(inp):
    ws = {}
    for l in range(4):
        din, dout = WIDTH[l], WIDTH[l + 1]
        coef = np.asarray(inp[f"coef{l}"], dtype=np.float32)
        sb = np.asarray(inp[f"sb{l}"], dtype=np.float32)
        ss = np.asarray(inp[f"ss{l}"], dtype=np.float32)
        spline_w = coef * ss[:, :, None] * (1.0 / 6.0)  # [din, dout, 8]
        nic = din // 128
        sp = spline_w.reshape(nic, 128, dout, NG).transpose(0, 3, 1, 2)
        base = sb.reshape(nic, 128, dout)[:, None]
        W = np.concatenate([sp, base], axis=1).reshape(nic * NB * 128, dout)
        ws[f"w{l}"] = np.ascontiguousarray(W).astype(ml_dtypes.bfloat16)
    return ws


def _run(inputs, trace=False, **kwargs):
    inp = {k: np.asarray(v) for k, v in inputs.items()}
    ws = _build_weights(inp)
    x = np.concatenate(
        [inp["inputs_y"].astype(np.float32), inp["inputs_u"].astype(np.float32)],
        axis=1,
    )
    xT = np.ascontiguousarray(x.T)  # [512 feat, 4096 batch]
    nc = _get_nc()
    in_maps = []
    for c in range(NCORES):
        m = {"xT": np.ascontiguousarray(xT[:, c * BPC : (c + 1) * BPC])}
        m.update(ws)
        in_maps.append(m)
    res = run_bass_kernel_spmd(
        nc, in_maps, core_ids=list(range(NCORES)), trace=trace, **kwargs
    )
    out = np.concatenate([r["out"] for r in res.results], axis=0)
    return out.astype(np.float32), res


def kernel(**inputs) -> np.ndarray:
    out, _ = _run(inputs)
    return out
